# revision 4
# baseline (speedup 1.0000x reference)
"""CosineTripletLoss Trainium2 kernel — 8-core data-parallel.

Math (per reference.py): loss = mean_i relu(margin - pos_i + sim[i, neg_idx_i])
where neg_idx_i = argmax_j of sim masked at the diagonal and wherever
sim > pos.  We compute t = sim - pos on-chip; then the per-row loss is
relu(margin + max_valid(t)) which needs no gather.  The reference's
all-masked fallback (argmax of an all(-1) row returns 0 -> neg = sim[i,0])
is reproduced via a per-row select on t[:, global j=0].

Sharding: rows of x split across 8 cores (1024 each).  y is replicated but
ROTATED per core (np.roll by -1024*core) so the diagonal of each core's
sim shard lands at local column == local row, letting all cores run the
same program.

Device pipeline per core:
  - cast x,y f32->fp16 during DMA (SWDGE), bounce through DRAM, and read
    back transposed (HW DMA transpose) to get the [d, row] layouts the PE
    needs for sim = x @ y^T.
  - 1024 fp16 matmuls (N=512, K accumulated 8x128) into PSUM.
  - ScalarE: t = sim - pos (per-partition bias), fp16 to SBUF.
  - VectorE: penalty mask (t>0 -> -8), diagonal -8, running elementwise max.
  - Final row-max, all-masked select, relu(margin + .), row sums.
Output: [128, 1] f32 partial sums per core; host sums / 8192.
"""

import json

import numpy as np

import concourse.bass as bass
import concourse.mybir as mybir
import concourse.tile as tile
from concourse import bass_utils

F32 = mybir.dt.float32
FP16 = mybir.dt.float16
ALU = mybir.AluOpType

N, D = 8192, 1024
NCORES = 8
R = N // NCORES          # 1024 rows per core
IB = R // 128            # 8 i-blocks
DB = D // 128            # 8 d-blocks
CHUNK = 1024             # y rows per stream chunk
NCH = N // CHUNK         # 8 chunks
JG = CHUNK // 128        # 8 row-groups per chunk
MARGIN = 0.05
PEN = -8.0               # penalty separating invalid (t>0) candidates
RM_INIT = -30.0
ALLMASK_THRESH = -3.0


# ---- workaround: this walrus accepts only ONE sem-wait per instruction ----
def _split_waits(bir: dict, maxw: int = 1) -> dict:
    nid = 0
    for fn in bir["functions"]:
        for blk in fn["blocks"]:
            new_insts = []
            for ins in blk["instructions"]:
                si = ins.get("sync_info") or {}
                ow = si.get("on_wait") or []
                if len(ow) > maxw:
                    extra = ow[:-maxw]
                    si["on_wait"] = ow[-maxw:]
                    for i in range(0, len(extra), maxw):
                        nid += 1
                        new_insts.append({
                            "debug": ins.get("debug", 0),
                            "engine": ins["engine"],
                            "ins": [], "outs": [],
                            "name": f"WSPLIT-{nid}",
                            "opcode": "NoOp",
                            "sync_info": {"on_update": [],
                                          "on_wait": extra[i:i + maxw]},
                        })
                new_insts.append(ins)
            blk["instructions"] = new_insts
    return bir


def _install_waitfix():
    import concourse.bass2jax as bass2jax
    if getattr(bass2jax, "_waitfix_installed", False):
        return
    orig = bass_utils.compile_bir_kernel

    def patched(bir_json, tmpdir, neff_name="file.neff"):
        bir = _split_waits(json.loads(bir_json))
        return orig(json.dumps(bir).encode(), tmpdir, neff_name)

    bass2jax.compile_bir_kernel = patched
    bass2jax._waitfix_installed = True


def build_kernel() -> bass.Bass:
    nc = bass.Bass("TRN2", debug=False)
    x_t = nc.dram_tensor("x", [R, D], F32, kind="ExternalInput")
    yr_t = nc.dram_tensor("yr", [N, D], F32, kind="ExternalInput")
    y0b_t = nc.dram_tensor("y0b", [128, D], F32, kind="ExternalInput")
    out_t = nc.dram_tensor("out", [128, 1], F32, kind="ExternalOutput")
    x16d = nc.dram_tensor("x16d", [R, D], FP16, kind="Internal")
    y16d = nc.dram_tensor("y16d", [N, D], FP16, kind="Internal")
    x = x_t.ap()
    yr = yr_t.ap()
    y16 = y16d.ap()

    with tile.TileContext(nc) as tc:
        with (
            tc.tile_pool(name="xt", bufs=1) as xt_pool,
            tc.tile_pool(name="x16p", bufs=1) as x16_pool,
            tc.tile_pool(name="yt", bufs=2) as yt_pool,
            tc.tile_pool(name="stage", bufs=4) as stage,
            tc.tile_pool(name="sp", bufs=3) as sp,
            tc.tile_pool(name="maccp", bufs=1) as maccp,
            tc.tile_pool(name="small", bufs=1) as small,
            tc.tile_pool(name="psum", bufs=4, space="PSUM") as psum_pool,
        ):
            # --- x: cast to fp16, bounce via DRAM, read back transposed ---
            x16 = []
            for ig in range(IB):
                t = x16_pool.tile([128, D], FP16, tag=f"x16_{ig}")
                nc.gpsimd.dma_start(out=t, in_=x[ig * 128:(ig + 1) * 128, :])
                nc.scalar.dma_start(out=x16d.ap()[ig * 128:(ig + 1) * 128, :],
                                    in_=t)
                x16.append(t)
            xT = []
            for db in range(DB):
                t = xt_pool.tile([128, R], FP16, tag=f"xT{db}")
                nc.sync.dma_start_transpose(
                    out=t, in_=x16d.ap()[:, db * 128:(db + 1) * 128])
                xT.append(t)

            # --- constants ---
            diagneg = small.tile([128, 128], FP16)
            nc.vector.memset(diagneg, 0.0)
            nc.gpsimd.affine_select(
                out=diagneg, in_=diagneg, compare_op=ALU.not_equal,
                fill=PEN, base=0, pattern=[[-1, 128]], channel_multiplier=1)

            y0bf = small.tile([128, D], F32)
            nc.sync.dma_start(out=y0bf, in_=y0b_t.ap())
            y0b = small.tile([128, D], FP16)
            nc.vector.tensor_copy(y0b, y0bf)

            pos_all = small.tile([128, IB], F32)
            negpos = small.tile([128, IB], F32)
            sim0 = small.tile([128, IB], F32)
            t0_all = small.tile([128, IB], F32)
            macc = [maccp.tile([128, CHUNK], FP16, tag=f"macc{ib}",
                               name=f"macc{ib}") for ib in range(IB)]

            for jc in range(NCH):
                # --- prep: cast chunk to fp16 in DRAM ---
                for jg in range(JG):
                    r0 = jc * CHUNK + jg * 128
                    st = stage.tile([128, D], FP16, tag="y16st")
                    nc.gpsimd.dma_start(out=st, in_=yr[r0:r0 + 128, :])
                    nc.scalar.dma_start(out=y16[r0:r0 + 128, :], in_=st)
                    if jc == 0:
                        # pos for i-block jg: rows of x and y coincide after
                        # the per-core rotation of y.
                        pr = sp.tile([128, D], FP16, tag="s")
                        nc.vector.tensor_mul(pr, x16[jg], st)
                        nc.vector.reduce_sum(pos_all[:, jg:jg + 1], pr,
                                             axis=mybir.AxisListType.X)
                if jc == 0:
                    nc.vector.tensor_scalar_mul(negpos, pos_all, -1.0)
                    for ig in range(IB):
                        pr = sp.tile([128, D], FP16, tag="s")
                        nc.vector.tensor_mul(pr, x16[ig], y0b)
                        nc.vector.reduce_sum(sim0[:, ig:ig + 1], pr,
                                             axis=mybir.AxisListType.X)
                    nc.vector.tensor_sub(t0_all, sim0, pos_all)

                # --- transposed read of the chunk ---
                yT = []
                for db in range(DB):
                    t = yt_pool.tile([128, CHUNK], FP16, tag=f"yT{db}")
                    nc.sync.dma_start_transpose(
                        out=t,
                        in_=y16[jc * CHUNK:(jc + 1) * CHUNK,
                                db * 128:(db + 1) * 128])
                    yT.append(t)

                # --- GEMM + mask + running max ---
                for ib in range(IB):
                    ps = psum_pool.tile([128, CHUNK], F32, tag="ps")
                    # db outer: each stationary xT tile is loaded once and
                    # streams both 512-wide rhs tiles before the next load.
                    for db in range(DB):
                        for jt in range(CHUNK // 512):
                            nc.tensor.matmul(
                                ps[:, jt * 512:(jt + 1) * 512],
                                lhsT=xT[db][:, ib * 128:(ib + 1) * 128],
                                rhs=yT[db][:, jt * 512:(jt + 1) * 512],
                                start=(db == 0), stop=(db == DB - 1))
                    s = sp.tile([128, CHUNK], FP16, tag="s")
                    nc.scalar.activation(
                        s, ps, mybir.ActivationFunctionType.Identity,
                        bias=negpos[:, ib:ib + 1], scale=1.0)
                    pen = sp.tile([128, CHUNK], FP16, tag="pen")
                    nc.vector.tensor_scalar(pen, s, 0.0, PEN,
                                            ALU.is_gt, ALU.mult)
                    if jc == 0:
                        nc.vector.tensor_add(
                            pen[:, ib * 128:(ib + 1) * 128],
                            pen[:, ib * 128:(ib + 1) * 128], diagneg)
                        v = macc[ib]
                        nc.vector.tensor_add(v, s, pen)
                    else:
                        v = sp.tile([128, CHUNK], FP16, tag="v")
                        nc.vector.tensor_add(v, s, pen)
                        nc.vector.tensor_max(macc[ib], macc[ib], v)

            # --- finals ---
            rm = small.tile([128, IB], F32)
            for ib in range(IB):
                nc.vector.reduce_max(rm[:, ib:ib + 1], macc[ib],
                                     axis=mybir.AxisListType.X)
            cm = small.tile([128, IB], F32)
            nc.vector.tensor_scalar(cm, rm, ALLMASK_THRESH, 0.0,
                                    ALU.is_lt, ALU.bypass)
            dm = small.tile([128, IB], F32)
            nc.vector.tensor_sub(dm, t0_all, rm)
            cd = small.tile([128, IB], F32)
            nc.vector.tensor_mul(cd, cm, dm)
            fin = small.tile([128, IB], F32)
            nc.vector.tensor_add(fin, rm, cd)
            lr = small.tile([128, IB], F32)
            nc.vector.tensor_scalar(lr, fin, MARGIN, 0.0, ALU.add, ALU.max)
            rs = small.tile([128, 1], F32)
            nc.vector.reduce_sum(rs, lr, axis=mybir.AxisListType.X)
            nc.scalar.dma_start(out=out_t.ap(), in_=rs)
    return nc


_NC_CACHE = None


def kernel(x: np.ndarray, y: np.ndarray) -> np.ndarray:
    global _NC_CACHE
    _install_waitfix()
    x = np.ascontiguousarray(x, dtype=np.float32)
    y = np.ascontiguousarray(y, dtype=np.float32)
    if _NC_CACHE is None:
        _NC_CACHE = build_kernel()
    nc = _NC_CACHE
    y0b = np.ascontiguousarray(np.broadcast_to(y[0:1, :], (128, D)),
                               dtype=np.float32)
    in_maps = []
    for c in range(NCORES):
        in_maps.append({
            "x": x[c * R:(c + 1) * R],
            "yr": np.ascontiguousarray(np.roll(y, -c * R, axis=0)),
            "y0b": y0b,
        })
    res = bass_utils.run_bass_kernel_spmd(nc, in_maps,
                                          core_ids=list(range(NCORES)))
    total = 0.0
    for c in range(NCORES):
        total += float(res.results[c]["out"].sum())
    return np.float32(total / N)
